# revision 1
# baseline (speedup 1.0000x reference)
"""DiffAttn (Differential Transformer attention) on 8 trn2 NeuronCores.

Sharding: tensor-parallel over heads. 16 heads / 8 cores = 2 heads per core
(= 4 of the 32 attention streams). Wq/Wk/Wv column-sharded (256 cols/core),
Wo + x replicated. The reference's "reshape without transposing heads back"
maps output row i = h*128 + (t//16) entirely to head h, so each core
produces rows [256*m, 256*(m+1)) of the final (2048, 2048) output with NO
collectives.

Per-core pipeline (all hot matmuls in float32r = 1 PE cycle/row):
  A) x -> xT via PE transpose; QT/KT (transposed) + V (natural) projections
  B) per (head, q-block of 512): causal logits (K=64), masked exp on ACT
     with row-sum accumulators, D = E0 - (lam*s0/s1)*E1 fused on DVE,
     PE-transpose D chunks -> PV transposed (N=512), LayerNorm via
     transpose-back, scaled by (1 - lambda_init)
  C) out-proj with stride-16 stationary slices of the transposed, normalized
     attention output (implements the scrambled reshape for free)
"""
import math
from contextlib import ExitStack

import numpy as np

import concourse.bass as bass
import concourse.mybir as mybir
import concourse.tile as tile
from concourse import bacc
from concourse import dve_ops
from concourse.bass_utils import run_bass_kernel_spmd
from concourse.masks import make_identity, make_causal_mask

F32 = mybir.dt.float32
F32R = mybir.dt.float32r
BF16 = mybir.dt.bfloat16
AF = mybir.ActivationFunctionType
OP = mybir.AluOpType

T = 2048
D_EMB = 2048
HD = 64           # head dim per stream
DV = 128          # value dim per head (2*HD)
N_CORES = 8
H_LOC = 2         # heads per core
SCALE = HD ** -0.5
LAMBDA_INIT = 0.8 - 0.6 * math.exp(-0.3 * 12)
LN_EPS = 1e-5
NEG = -1e30

TB = 4            # t-blocks of 512 in phase A
QB = 4            # q-blocks of 512 in phase B
KD = 16           # contraction chunks of 128 over D_EMB


def _build_program():
    nc = bacc.Bacc("TRN2", target_bir_lowering=False, debug=False)

    x_d = nc.dram_tensor("x", [T, D_EMB], F32R, kind="ExternalInput").ap()
    wq_d = nc.dram_tensor("wq", [D_EMB, 256], F32R, kind="ExternalInput").ap()
    wk_d = nc.dram_tensor("wk", [D_EMB, 256], F32R, kind="ExternalInput").ap()
    wv_d = nc.dram_tensor("wv", [D_EMB, 256], F32R, kind="ExternalInput").ap()
    wo_d = nc.dram_tensor("wo", [D_EMB, D_EMB], F32R, kind="ExternalInput").ap()
    neglam_d = nc.dram_tensor("neglam", [128, 1], F32, kind="ExternalInput").ap()
    y_d = nc.dram_tensor("y", [256, D_EMB], F32, kind="ExternalOutput").ap()

    with tile.TileContext(nc) as tc:
        with ExitStack() as ctx:
            const = ctx.enter_context(tc.tile_pool(name="const", bufs=1))
            qkv = ctx.enter_context(tc.tile_pool(name="qkv", bufs=1))
            mtp = ctx.enter_context(tc.tile_pool(name="mtp", bufs=1))

            ident = const.tile([128, 128], F32)
            make_identity(nc, ident[:])
            ident_r = const.tile([128, 128], F32R)
            nc.scalar.copy(ident_r[:], ident[:])
            neglam = const.tile([128, 1], F32)
            nc.sync.dma_start(neglam[:], neglam_d)
            eps_t = const.tile([128, 1], F32)
            nc.gpsimd.memset(eps_t[:], LN_EPS)

            # single [128,128] additive causal mask (0 on/below diag, NEG above)
            cmask = const.tile([128, 128], F32)
            make_causal_mask(nc, cmask[:], mask_val=NEG)

            # persistent tensors
            QT = [qkv.tile([128, T], F32R, name=f"qt{h}") for h in range(H_LOC)]
            KT = [qkv.tile([128, T], F32R, name=f"kt{h}") for h in range(H_LOC)]
            V = [qkv.tile([128, 256], F32R, name=f"v{t}") for t in range(16)]
            MT = [mtp.tile([128, T], F32R, name=f"mt{h}") for h in range(H_LOC)]

            # ---------------- Phase A: xT + projections ----------------
            with ExitStack() as actx:
                wpool = actx.enter_context(tc.tile_pool(name="wpool", bufs=1))
                xpool = actx.enter_context(tc.tile_pool(name="xpool", bufs=1))
                xtpool = actx.enter_context(tc.tile_pool(name="xtpool", bufs=1))
                ps_t = actx.enter_context(
                    tc.tile_pool(name="ps_t", bufs=2, space="PSUM"))
                ps_qk = actx.enter_context(
                    tc.tile_pool(name="ps_qk", bufs=2, space="PSUM"))
                ps_v = actx.enter_context(
                    tc.tile_pool(name="ps_v", bufs=2, space="PSUM"))

                wq_t = wpool.tile([128, KD, 256], F32R, name="wq")
                wk_t = wpool.tile([128, KD, 256], F32R, name="wk")
                wv_t = wpool.tile([128, KD, 256], F32R, name="wv")
                for dj in range(KD):
                    nc.sync.dma_start(wq_t[:, dj, :], wq_d[dj * 128:(dj + 1) * 128, :])
                    nc.sync.dma_start(wk_t[:, dj, :], wk_d[dj * 128:(dj + 1) * 128, :])
                    nc.sync.dma_start(wv_t[:, dj, :], wv_d[dj * 128:(dj + 1) * 128, :])

                for tb in range(TB):
                    xt_blk = []
                    x_tiles = []
                    for tt in range(4):
                        xt_ = xpool.tile([128, D_EMB], F32R, name=f"x{tt}")
                        nc.sync.dma_start(
                            xt_[:], x_d[tb * 512 + tt * 128: tb * 512 + (tt + 1) * 128, :])
                        x_tiles.append(xt_)
                    for dj in range(KD):
                        pst = ps_t.tile([128, 512], F32R, name="pst")
                        for tt in range(4):
                            nc.tensor.transpose(
                                pst[:, tt * 128:(tt + 1) * 128],
                                x_tiles[tt][:, dj * 128:(dj + 1) * 128], ident_r[:])
                        xt_c = xtpool.tile([128, 512], F32R, name=f"xt{dj}")
                        nc.vector.tensor_copy(xt_c[:], pst[:])
                        xt_blk.append(xt_c)

                    # QT / KT: out [128 dq(head h), 512 t]
                    for h in range(H_LOC):
                        for w_t, dst in ((wq_t, QT), (wk_t, KT)):
                            psq = ps_qk.tile([128, 512], F32, name="psqk")
                            for dj in range(KD):
                                nc.tensor.matmul(
                                    psq[:],
                                    w_t[:, dj, h * 128:(h + 1) * 128],
                                    xt_blk[dj][:],
                                    start=(dj == 0), stop=(dj == KD - 1))
                            nc.scalar.copy(
                                dst[h][:, tb * 512:(tb + 1) * 512], psq[:])
                    # V: out [128 t, 256 dv]
                    for tt in range(4):
                        psv = ps_v.tile([128, 256], F32, name="psv")
                        for dj in range(KD):
                            nc.tensor.matmul(
                                psv[:],
                                xt_blk[dj][:, tt * 128:(tt + 1) * 128],
                                wv_t[:, dj, :],
                                start=(dj == 0), stop=(dj == KD - 1))
                        nc.scalar.copy(V[tb * 4 + tt][:], psv[:])

            # Wo prefetch pool opened early so phase-C DMAs overlap phase B
            wopool = ctx.enter_context(tc.tile_pool(name="wopool", bufs=10))

            # ---------------- Phase B: attention + LN ----------------
            with ExitStack() as bctx:
                etpool = bctx.enter_context(tc.tile_pool(name="etpool", bufs=5))
                upool = bctx.enter_context(tc.tile_pool(name="upool", bufs=2))
                smpool = bctx.enter_context(tc.tile_pool(name="smpool", bufs=2))
                ps_st0 = bctx.enter_context(
                    tc.tile_pool(name="ps_st0", bufs=2, space="PSUM"))
                ps_st1 = bctx.enter_context(
                    tc.tile_pool(name="ps_st1", bufs=2, space="PSUM"))
                ps_ut0 = bctx.enter_context(
                    tc.tile_pool(name="ps_ut0", bufs=1, space="PSUM"))
                ps_ut1 = bctx.enter_context(
                    tc.tile_pool(name="ps_ut1", bufs=1, space="PSUM"))
                ps_sum = bctx.enter_context(
                    tc.tile_pool(name="ps_sum", bufs=1, space="PSUM"))
                ps_misc = bctx.enter_context(
                    tc.tile_pool(name="ps_misc", bufs=1, space="PSUM"))

                # f32r ones for the column-sum and broadcast matmuls
                ones_col = const.tile([128, 1], F32R)
                ones_row = const.tile([1, 128], F32R)
                o32 = const.tile([128, 1], F32)
                nc.gpsimd.memset(o32[:], 1.0)
                nc.scalar.copy(ones_col[:], o32[:])
                o32r = const.tile([1, 128], F32)
                nc.gpsimd.memset(o32r[:], 1.0)
                nc.scalar.copy(ones_row[:], o32r[:])
                lnc_t = const.tile([128, 1], F32)
                nc.gpsimd.memset(lnc_t[:], math.log(1.0 - LAMBDA_INIT))

                # transposed causal masks per diag position j:
                # ST[x(k), y(q)]: pass 0 where j*128 + x - y <= 0, else NEG
                maskT = []
                for j in range(QB):
                    mk = const.tile([128, 512], F32, name=f"maskT{j}")
                    nc.gpsimd.memset(mk[:], 1.0)
                    # keep 1 where k<=q: (y - x - j*128) >= 0; else fill 0
                    nc.gpsimd.affine_select(
                        out=mk[:], in_=mk[:], compare_op=OP.is_ge, fill=0.0,
                        base=-j * 128, pattern=[[1, 512]], channel_multiplier=-1,
                    )
                    maskT.append(mk)

                for h in range(H_LOC):
                    for qb in range(QB):
                        nck = 4 * qb + 4
                        sum_ps = [ps_sum.tile([1, 512], F32, name="ssum"),
                                  ps_misc.tile([1, 512], F32, name="misc")]
                        ut_ps = [ps_ut0.tile([128, 512], F32, name="ut0"),
                                 ps_ut1.tile([128, 512], F32, name="ut1")]
                        pending = []  # (kc, s, et) awaiting sums/PV
                        for kc in range(nck):
                            for s, pool_s in ((0, ps_st0), (1, ps_st1)):
                                st_ps = pool_s.tile([128, 512], F32,
                                                    name=f"st{s}")
                                nc.tensor.matmul(
                                    st_ps[:],
                                    KT[h][s * 64:(s + 1) * 64,
                                          kc * 128:(kc + 1) * 128],
                                    QT[h][s * 64:(s + 1) * 64,
                                          qb * 512:(qb + 1) * 512],
                                    start=True, stop=True)
                                et = etpool.tile([128, 512], F32R,
                                                 name=f"et{s}")
                                nc.scalar.activation(et[:], st_ps[:],
                                                     AF.Exp, scale=SCALE)
                                if kc >= 4 * qb:  # diagonal region: 0/1 mask
                                    j = kc - 4 * qb
                                    nc.vector.tensor_tensor(
                                        et[:], et[:], maskT[j][:], OP.mult)
                                pending.append((kc, s, et))
                            while len(pending) > 4:
                                pkc, ps_, pet = pending.pop(0)
                                nc.tensor.matmul(
                                    sum_ps[ps_][:], ones_col[:], pet[:],
                                    start=(pkc == 0), stop=(pkc == nck - 1))
                                nc.tensor.matmul(
                                    ut_ps[ps_][:],
                                    V[pkc][:, h * 128:(h + 1) * 128], pet[:],
                                    start=(pkc == 0), stop=(pkc == nck - 1))
                        for pkc, ps_, pet in pending:
                            nc.tensor.matmul(
                                sum_ps[ps_][:], ones_col[:], pet[:],
                                start=(pkc == 0), stop=(pkc == nck - 1))
                            nc.tensor.matmul(
                                ut_ps[ps_][:],
                                V[pkc][:, h * 128:(h + 1) * 128], pet[:],
                                start=(pkc == 0), stop=(pkc == nck - 1))

                        # negc_row = -lam * s0 / s1  (per q column)
                        s0_sb = smpool.tile([1, 512], F32, name="s0sb")
                        nc.scalar.copy(s0_sb[:], sum_ps[0][:])
                        r1 = smpool.tile([1, 512], F32, name="r1")
                        nc.vector.reciprocal(r1[:], sum_ps[1][:])
                        t0 = smpool.tile([1, 512], F32, name="t0")
                        nc.vector.tensor_tensor(t0[:], s0_sb[:], r1[:],
                                                OP.mult)
                        negc_row = smpool.tile([1, 512], F32R, name="negc")
                        nc.vector.tensor_scalar(negc_row[:], t0[:],
                                                neglam[0:1, 0:1], None, OP.mult)
                        cb_ps = ps_misc.tile([128, 512], F32, name="misc")
                        nc.tensor.matmul(cb_ps[:], ones_row[:], negc_row[:],
                                         start=True, stop=True)
                        ut1_sb = upool.tile([128, 512], F32, name="ut1_sb")
                        nc.vector.tensor_copy(ut1_sb[:], ut_ps[1][:])
                        tmp = upool.tile([128, 512], F32, name="tmp")
                        nc.vector.tensor_tensor(tmp[:], cb_ps[:], ut1_sb[:],
                                                OP.mult)
                        ut_sb = upool.tile([128, 512], F32R, name="ut_sb")
                        nc.vector.tensor_tensor(ut_sb[:], ut_ps[0][:], tmp[:],
                                                OP.add)

                        # LayerNorm over dv via transpose-back
                        sums = smpool.tile([128, 4], F32, name="lnsum")
                        sumsq = smpool.tile([128, 4], F32, name="lnsumsq")
                        u_ps = ps_misc.tile([128, 512], F32R, name="misc")
                        for qj in range(QB):
                            nc.tensor.transpose(
                                u_ps[:, qj * 128:(qj + 1) * 128],
                                ut_sb[:, qj * 128:(qj + 1) * 128], ident_r[:])
                        u_sbs = []
                        for qj in range(QB):
                            sl = slice(qj * 128, (qj + 1) * 128)
                            u_sb = upool.tile([128, 128], F32, name=f"u{qj}")
                            nc.scalar.activation(
                                u_sb[:], u_ps[:, sl], AF.Copy,
                                accum_out=sums[:, qj:qj + 1])
                            usq = upool.tile([128, 128], F32, name="usq")
                            nc.scalar.activation(
                                usq[:], u_ps[:, sl], AF.Square,
                                accum_out=sumsq[:, qj:qj + 1])
                            u_sbs.append(u_sb)
                        mu = smpool.tile([128, 4], F32, name="mu")
                        nc.vector.tensor_scalar(mu[:], sums[:], 1.0 / DV, None,
                                                OP.mult)
                        musq = smpool.tile([128, 4], F32, name="musq")
                        nc.vector.tensor_tensor(musq[:], mu[:], mu[:], OP.mult)
                        var = smpool.tile([128, 4], F32, name="var")
                        nc.vector.tensor_scalar(var[:], sumsq[:], 1.0 / DV, None,
                                                OP.mult)
                        nc.vector.tensor_tensor(var[:], var[:], musq[:],
                                                OP.subtract)
                        # rstd' = (1-li)/sqrt(var+eps) = exp(-.5*ln(var+eps)+ln(1-li))
                        lnv = smpool.tile([128, 4], F32, name="lnv")
                        nc.scalar.activation(lnv[:], var[:], AF.Ln, bias=eps_t[:])
                        rstdp = smpool.tile([128, 4], F32, name="rstdp")
                        nc.scalar.activation(rstdp[:], lnv[:], AF.Exp,
                                             scale=-0.5, bias=lnc_t[:])
                        mt_ps = ps_misc.tile([128, 512], F32R, name="misc")
                        for qj in range(QB):
                            un = upool.tile([128, 128], F32R, name="un")
                            nc.vector.tensor_scalar(
                                un[:], u_sbs[qj][:], mu[:, qj:qj + 1],
                                rstdp[:, qj:qj + 1], OP.subtract, OP.mult)
                            nc.tensor.transpose(
                                mt_ps[:, qj * 128:(qj + 1) * 128], un[:], ident_r[:])
                        nc.scalar.copy(
                            MT[h][:, qb * 512:(qb + 1) * 512], mt_ps[:])

            # ---------------- Phase C: out-proj ----------------
            with ExitStack() as cctx:
                ypool = cctx.enter_context(tc.tile_pool(name="ypool", bufs=2))
                ps_y = cctx.enter_context(
                    tc.tile_pool(name="ps_y", bufs=2, space="PSUM"))
                mt_r = [MT[h][:].rearrange("p (m g) -> p g m", g=16)
                        for h in range(H_LOC)]
                for do in range(4):
                    ys = [ps_y.tile([128, 512], F32, name=f"y{h}")
                          for h in range(H_LOC)]
                    for j in range(KD):
                        wo_t = wopool.tile([128, 512], F32R, name="wo")
                        nc.sync.dma_start(
                            wo_t[:],
                            wo_d[j * 128:(j + 1) * 128, do * 512:(do + 1) * 512])
                        for h in range(H_LOC):
                            nc.tensor.matmul(
                                ys[h][:], mt_r[h][:, j, :], wo_t[:],
                                start=(j == 0), stop=(j == KD - 1))
                    for h in range(H_LOC):
                        y_sb = ypool.tile([128, 512], F32, name="ysb")
                        nc.scalar.copy(y_sb[:], ys[h][:])
                        nc.scalar.dma_start(
                            y_d[h * 128:(h + 1) * 128, do * 512:(do + 1) * 512],
                            y_sb[:])

    nc.compile()
    return nc


_NC_CACHE = None


def kernel(x, Wq, Wk, Wv, Wo, lambda_q1, lambda_k1, lambda_q2, lambda_k2):
    global _NC_CACHE
    if _NC_CACHE is None:
        _NC_CACHE = _build_program()
    nc = _NC_CACHE

    x = np.asarray(x, dtype=np.float32)
    B = x.shape[0]
    x2 = np.ascontiguousarray(x.reshape(T, D_EMB))
    Wq = np.asarray(Wq, np.float32)
    Wk = np.asarray(Wk, np.float32)
    Wv = np.asarray(Wv, np.float32)
    Wo = np.ascontiguousarray(np.asarray(Wo, np.float32))

    lam = (math.exp(float(np.dot(np.asarray(lambda_q1, np.float64),
                                 np.asarray(lambda_k1, np.float64))))
           - math.exp(float(np.dot(np.asarray(lambda_q2, np.float64),
                                   np.asarray(lambda_k2, np.float64))))
           + LAMBDA_INIT)
    neglam = np.full((128, 1), -lam, dtype=np.float32)

    in_maps = []
    for m in range(N_CORES):
        sl = slice(256 * m, 256 * (m + 1))
        in_maps.append({
            "x": x2,
            "wq": np.ascontiguousarray(Wq[:, sl]),
            "wk": np.ascontiguousarray(Wk[:, sl]),
            "wv": np.ascontiguousarray(Wv[:, sl]),
            "wo": Wo,
            "neglam": neglam,
        })

    res = run_bass_kernel_spmd(nc, in_maps, list(range(N_CORES)))
    y = np.concatenate([res.results[m]["y"] for m in range(N_CORES)], axis=0)
    return y.reshape(B, T, D_EMB)



# revision 15
# speedup vs baseline: 1.3114x; 1.3114x over previous
"""DiffAttn (Differential Transformer attention) on 8 trn2 NeuronCores.

Sharding: tensor-parallel over heads. 16 heads / 8 cores = 2 heads per core.
Wq/Wk/Wv column-sharded (256 cols/core), Wo + x replicated. The reference's
"reshape without transposing heads back" maps output row r = h*128 + (t//16)
entirely to head h, so each core produces rows [256*m, 256*(m+1)) of the
final (2048, 2048) output with NO collectives.

v2 design notes (perf):
- Phase A (xT via PE transpose + Q/K/V projections, f32r) is woven with
  phase B (attention) at emission time so the PE queue never drains: the
  TRN2 PE p-state only reaches 2.4 GHz after ~3us of continuous busy.
- Q^T/K^T stored fp16; logits matmuls fp16 (1 cyc/row at any N, enabling
  exact causal column trimming).
- PV uses stationary=exp-tile chunk [128k,128q], moving=V||ones [128k,129]:
  output lands natural [q, dv] with softmax row-sums free in column 128.
  No ones-matmuls, no [1,512] reciprocals, no LN transpose round trip.
- Diff d = E0V - (lam*s0/s1)*E1V via one fused scalar_tensor_tensor per
  q-subtile (free row-sum accumulation for the LN mean); sum(d^2) via one
  tensor_tensor_reduce. LayerNorm rstd for all 32 tiles computed with ONE
  Ln + ONE Exp (2 ACT table loads total).
- Phase B.5 normalizes + PE-transposes to MT fp16; phase C out-proj in fp16
  with Wo host-cast to fp16 and prefetched.
"""
import math
from contextlib import ExitStack

import numpy as np

import concourse.bass as bass
import concourse.mybir as mybir
import concourse.tile as tile
from concourse import bacc
from concourse.bass_utils import run_bass_kernel_spmd
from concourse.masks import make_identity

F32 = mybir.dt.float32
F32R = mybir.dt.float32r
F16 = mybir.dt.float16
AF = mybir.ActivationFunctionType
OP = mybir.AluOpType

T = 2048
D_EMB = 2048
HD = 64           # head dim per stream
N_CORES = 8
H_LOC = 2         # heads per core
SCALE = HD ** -0.5
LAMBDA_INIT = 0.8 - 0.6 * math.exp(-0.3 * 12)
LN_EPS = 1e-5

TB = 4            # t-blocks of 512 in phase A
QB = 4            # q-blocks of 512 in phase B
KD = 16           # contraction chunks of 128 over D_EMB


def _build_program():
    nc = bacc.Bacc("TRN2", target_bir_lowering=False, debug=False)

    x_d = nc.dram_tensor("x", [T, D_EMB], F32R, kind="ExternalInput").ap()
    wq_d = nc.dram_tensor("wq", [D_EMB, 256], F32R, kind="ExternalInput").ap()
    wk_d = nc.dram_tensor("wk", [D_EMB, 256], F32R, kind="ExternalInput").ap()
    wv_d = nc.dram_tensor("wv", [D_EMB, 256], F32R, kind="ExternalInput").ap()
    wo_d = nc.dram_tensor("wo", [D_EMB, D_EMB], F16, kind="ExternalInput").ap()
    neglam_d = nc.dram_tensor("neglam", [128, 1], F32, kind="ExternalInput").ap()
    y_d = nc.dram_tensor("y", [256, D_EMB], F32, kind="ExternalOutput").ap()

    with tile.TileContext(nc) as tc:
        with ExitStack() as ctx:
            const = ctx.enter_context(tc.tile_pool(name="const", bufs=1))
            qkv = ctx.enter_context(tc.tile_pool(name="qkv", bufs=1))
            dst = ctx.enter_context(tc.tile_pool(name="dst", bufs=1))
            small = ctx.enter_context(tc.tile_pool(name="small", bufs=3))
            etp = ctx.enter_context(tc.tile_pool(name="etp", bufs=6))
            ps = ctx.enter_context(tc.tile_pool(name="ps", bufs=2, space="PSUM"))
            ps_log = ctx.enter_context(
                tc.tile_pool(name="ps_log", bufs=2, space="PSUM"))
            ps_pv0 = ctx.enter_context(
                tc.tile_pool(name="ps_pv0", bufs=2, space="PSUM"))
            ps_pv1 = ctx.enter_context(
                tc.tile_pool(name="ps_pv1", bufs=2, space="PSUM"))

            # ---------------- constants ----------------
            ident = const.tile([128, 128], F32)
            make_identity(nc, ident[:])
            ident_r = const.tile([128, 128], F32R)
            nc.scalar.copy(ident_r[:], ident[:])
            ident_h = const.tile([128, 128], F16)
            nc.scalar.copy(ident_h[:], ident[:])
            neglam = const.tile([128, 1], F32)
            nc.scalar.dma_start(neglam[:], neglam_d)
            eps_t = const.tile([128, 1], F32)
            nc.gpsimd.memset(eps_t[:], LN_EPS)
            lnc_t = const.tile([128, 1], F32)
            nc.gpsimd.memset(lnc_t[:], math.log(1.0 - LAMBDA_INIT))
            ebias_t = const.tile([128, 1], F32)
            nc.gpsimd.memset(ebias_t[:], -6.0)
            # tri[k, q] = 1 where k <= q else 0 (causal keep-mask, fp16)
            tri = const.tile([128, 128], F16)
            nc.gpsimd.memset(tri[:], 1.0)
            nc.gpsimd.affine_select(
                out=tri[:], in_=tri[:], compare_op=OP.is_ge, fill=0.0,
                base=0, pattern=[[1, 128]], channel_multiplier=-1)

            # ---------------- persistent tensors ----------------
            QT = [qkv.tile([128, T], F16, name=f"qt{h}") for h in range(H_LOC)]
            KT = [qkv.tile([128, T], F16, name=f"kt{h}") for h in range(H_LOC)]
            # V[t]: [k(128), head(2), 132]; cols 0:128 = V data, col 128 = 1.0
            V = [qkv.tile([128, 2, 132], F16, name=f"v{t}") for t in range(16)]
            MT = [qkv.tile([128, T], F16, name=f"mt{h}") for h in range(H_LOC)]
            for t in range(16):
                nc.gpsimd.memset(V[t][:, :, 128:129], 1.0)

            # d staging + LN statistics (col = h*16 + qb*4 + qj)
            dtiles = [[dst.tile([128, 128], F16, name=f"d{h}_{i}")
                       for i in range(16)] for h in range(H_LOC)]
            dsums = dst.tile([128, 32], F32, name="dsums")
            sumsq = dst.tile([128, 32], F32, name="sumsq")
            mus = dst.tile([128, 32], F32, name="mus")
            rstd = dst.tile([128, 32], F32, name="rstd")

            # ---------------- phase B closures ----------------
            def gen_b_closures(h, qb):
                nck = 4 * qb + 4
                st_state = {}

                def mk_u1(kc):
                    def u1():
                        if kc == 0:
                            st_state["pv"] = [
                                [ps_pv0.tile([128, 2, 136], F32, name="pv0")
                                 for _ in range(2)],
                                [ps_pv1.tile([128, 2, 136], F32, name="pv1")
                                 for _ in range(2)],
                            ]
                        j = kc - 4 * qb
                        qs = 128 * j if j > 0 else 0
                        ets = []
                        for s in (0, 1):
                            stp = ps_log.tile([128, 512], F32, name="pslog")
                            nc.tensor.matmul(
                                stp[:, qs:512],
                                KT[h][s * 64:(s + 1) * 64,
                                      kc * 128:(kc + 1) * 128],
                                QT[h][s * 64:(s + 1) * 64,
                                      qb * 512 + qs:(qb + 1) * 512],
                                start=True, stop=True)
                            # bias -6 keeps exp and the E*V products in fp16
                            # range; the softmax ratio and LayerNorm are
                            # invariant to the uniform e^-6 factor
                            et = etp.tile([128, 512], F16, name="et")
                            nc.scalar.activation(et[:, qs:512], stp[:, qs:512],
                                                 AF.Exp, scale=SCALE,
                                                 bias=ebias_t[:])
                            if j >= 0:
                                nc.gpsimd.tensor_tensor(
                                    et[:, qs:qs + 128], et[:, qs:qs + 128],
                                    tri[:], OP.mult)
                            ets.append(et)
                        st_state[kc] = ets
                    return u1

                def mk_u2(kc):
                    def u2():
                        j = kc - 4 * qb
                        ets = st_state.pop(kc)
                        pv = st_state["pv"]
                        for s in (0, 1):
                            for qj in range(4):
                                if j > qj:
                                    continue
                                # start=True zeroes the WHOLE 2KB psum zero
                                # region, so only the first matmul into each
                                # bank starts; the odd-qj group's first write
                                # lands on pending-zero bytes and overwrites.
                                nc.tensor.matmul(
                                    pv[s][qj // 2][:, qj % 2, 0:129],
                                    ets[s][:, qj * 128:(qj + 1) * 128],
                                    V[kc][:, h, 0:129],
                                    start=(kc == 0 and qj % 2 == 0),
                                    stop=(kc == 4 * qb + qj),
                                    skip_group_check=True)
                    return u2

                def epi():
                    pv = st_state.pop("pv")
                    for qj in range(4):
                        col = h * 16 + qb * 4 + qj
                        p0 = pv[0][qj // 2][:, qj % 2, :]
                        p1 = pv[1][qj // 2][:, qj % 2, :]
                        # d = p0/s0 - lam*p1/s1 — matching the reference's
                        # softmax normalization exactly (so LN_EPS compares
                        # against the same variance scale, and the exp bias
                        # e^-6 cancels)
                        r1 = small.tile([128, 1], F32, name="r1")
                        nc.vector.reciprocal(r1[:], p1[:, 128:129])
                        r0 = small.tile([128, 1], F32, name="r0")
                        nc.vector.reciprocal(r0[:], p0[:, 128:129])
                        negc = small.tile([128, 1], F32, name="negc")
                        nc.vector.tensor_tensor(
                            negc[:], neglam[:], r1[:], OP.mult)
                        dt_ = dtiles[h][qb * 4 + qj]
                        tmp = small.tile([128, 128], F32, name="tmp")
                        nc.vector.tensor_scalar(
                            tmp[:], p1[:, 0:128], negc[:], None, OP.mult)
                        nc.vector.scalar_tensor_tensor(
                            dt_[:], p0[:, 0:128], r0[:], tmp[:],
                            op0=OP.mult, op1=OP.add,
                            accum_out=dsums[:, col:col + 1])
                        # (d * 1.0) * d with accumulated sum -> sum(d^2);
                        # native InstTensorScalarPtr (tensor_tensor_reduce is
                        # a custom-DVE op whose ucode table crashes this
                        # execution path on hardware)
                        dsq = small.tile([128, 128], F16, name="dsq")
                        nc.vector.scalar_tensor_tensor(
                            dsq[:], dt_[:], 1.0, dt_[:],
                            op0=OP.mult, op1=OP.mult,
                            accum_out=sumsq[:, col:col + 1])

                # lookahead order: u1(k+1) is emitted before u2(k) so the
                # exp of tile k finishes behind the logits matmuls of k+1
                us = [mk_u1(kc) for kc in range(nck)]
                vs = [mk_u2(kc) for kc in range(nck)]
                out = [us[0]]
                for kc in range(1, nck):
                    out.append(us[kc])
                    out.append(vs[kc - 1])
                out.append(vs[nck - 1])
                out.append(epi)
                return out

            def weave(quanta, bcl):
                n, m = len(bcl), max(1, len(quanta))
                bi = 0
                for i, q in enumerate(quanta):
                    q()
                    tgt = (i + 1) * n // m
                    while bi < tgt:
                        bcl[bi]()
                        bi += 1
                while bi < n:
                    bcl[bi]()
                    bi += 1

            # ---------------- phase A (woven with B) ----------------
            with ExitStack() as actx:
                wpool = actx.enter_context(tc.tile_pool(name="wpool", bufs=1))
                xpool = actx.enter_context(tc.tile_pool(name="xpool", bufs=2))
                xtc = actx.enter_context(tc.tile_pool(name="xtc", bufs=16))

                wq_t = wpool.tile([128, KD, 256], F32R, name="wq")
                wk_t = wpool.tile([128, KD, 256], F32R, name="wk")
                wv_t = wpool.tile([128, KD, 256], F32R, name="wv")
                # halves interleaved so early dj chunks of all three arrive
                # first (scalar queue; x tiles go on the sync queue)
                for lo, hi in ((0, 8), (8, 16)):
                    for w_t, w_d in ((wq_t, wq_d), (wk_t, wk_d), (wv_t, wv_d)):
                        nc.scalar.dma_start(
                            w_t[:, lo:hi, :],
                            w_d[lo * 128:hi * 128, :].rearrange(
                                "(a p) c -> p a c", p=128))

                for tb in range(TB):
                    x_t = xpool.tile([128, 4, D_EMB], F32R, name="xin")
                    xts = [xtc.tile([128, 512], F32R, name="xtc")
                           for _ in range(KD)]
                    quanta = []

                    def dma_q(x_t=x_t, tb=tb):
                        for tt in range(4):
                            nc.sync.dma_start(
                                x_t[:, tt, :],
                                x_d[tb * 512 + tt * 128:
                                    tb * 512 + (tt + 1) * 128, :])
                    quanta.append(dma_q)

                    for djp in range(8):
                        def tq(djp=djp, x_t=x_t, xts=xts):
                            for dj in (2 * djp, 2 * djp + 1):
                                pst = ps.tile([128, 512], F32R, name="ps")
                                for tt in range(4):
                                    nc.tensor.transpose(
                                        pst[:, tt * 128:(tt + 1) * 128],
                                        x_t[:, tt, dj * 128:(dj + 1) * 128],
                                        ident_r[:])
                                nc.vector.tensor_copy(xts[dj][:], pst[:])
                        quanta.append(tq)

                    for h in range(H_LOC):
                        for w_t, dstq in ((wq_t, QT), (wk_t, KT)):
                            def qk(w_t=w_t, dstq=dstq, h=h, tb=tb, xts=xts):
                                psq = ps.tile([128, 512], F32, name="ps")
                                for dj in range(KD):
                                    nc.tensor.matmul(
                                        psq[:],
                                        w_t[:, dj, h * 128:(h + 1) * 128],
                                        xts[dj][:],
                                        start=(dj == 0), stop=(dj == KD - 1))
                                nc.scalar.copy(
                                    dstq[h][:, tb * 512:(tb + 1) * 512],
                                    psq[:])
                            quanta.append(qk)

                    for tt in range(4):
                        def vq(tt=tt, tb=tb, xts=xts):
                            psv = ps.tile([128, 256], F32, name="ps")
                            for dj in range(KD):
                                nc.tensor.matmul(
                                    psv[:],
                                    xts[dj][:, tt * 128:(tt + 1) * 128],
                                    wv_t[:, dj, :],
                                    start=(dj == 0), stop=(dj == KD - 1))
                            vt = V[tb * 4 + tt]
                            nc.scalar.copy(
                                vt[:, :, 0:128],
                                psv[:].rearrange("p (h c) -> p h c", h=2))
                            # center V rows over dv: LN(d) is exactly
                            # invariant, but removes the near-constant row
                            # component that otherwise amplifies fp16
                            # rounding ~50x through the 1/sigma of rows
                            # where the two streams nearly cancel
                            vsum = small.tile([128, 2], F32, name="vsum")
                            nc.vector.tensor_reduce(
                                out=vsum[:], in_=vt[:, :, 0:128],
                                axis=mybir.AxisListType.X, op=OP.add)
                            nmean = small.tile([128, 2], F32, name="nmean")
                            nc.vector.tensor_scalar(
                                nmean[:], vsum[:], -1.0 / 128.0, None,
                                OP.mult)
                            for hh in range(H_LOC):
                                nc.vector.tensor_scalar(
                                    vt[:, hh, 0:128], vt[:, hh, 0:128],
                                    nmean[:, hh:hh + 1], None, OP.add)
                        quanta.append(vq)

                    # weave previous q-block's attention into this stage
                    bcl = []
                    if tb >= 1:
                        for h in range(H_LOC):
                            bcl += gen_b_closures(h, tb - 1)
                    weave(quanta, bcl)

            # ---------------- phase B tail: qb=3 + Wo prefetch ----------
            wopool = ctx.enter_context(tc.tile_pool(name="wopool", bufs=2))
            wo_tiles = {}

            def wo_dma(do):
                def f():
                    wo_t = wopool.tile([128, KD, 512], F16, name="wo")
                    nc.sync.dma_start(
                        wo_t[:],
                        wo_d[:, do * 512:(do + 1) * 512].rearrange(
                            "(a p) c -> p a c", p=128))
                    wo_tiles[do] = wo_t
                return f

            tail = gen_b_closures(0, 3)
            tail.insert(2, wo_dma(0))
            tail.insert(len(tail) // 2, wo_dma(1))
            tail += gen_b_closures(1, 3)
            for f in tail:
                f()

            # ---------------- phase B.5: LN + transpose to MT ----------
            musq = dst.tile([128, 32], F32, name="musq")
            varp = dst.tile([128, 32], F32, name="varp")
            nc.vector.tensor_scalar(mus[:], dsums[:], 1.0 / 128.0, None,
                                    OP.mult)
            nc.vector.tensor_tensor(musq[:], mus[:], mus[:], OP.mult)
            nc.vector.scalar_tensor_tensor(
                varp[:], sumsq[:], 1.0 / 128.0, musq[:],
                op0=OP.mult, op1=OP.subtract)
            lnv = dst.tile([128, 32], F32, name="lnv")
            nc.scalar.activation(lnv[:], varp[:], AF.Ln, bias=eps_t[:])
            # rstd' = (1-lambda_init) / sqrt(var+eps) = exp(-.5*lnv + lnc)
            nc.scalar.activation(rstd[:], lnv[:], AF.Exp, scale=-0.5,
                                 bias=lnc_t[:])

            for h in range(H_LOC):
                for qb in range(QB):
                    mt_ps = ps.tile([128, 512], F16, name="ps")
                    for qj in range(4):
                        i = qb * 4 + qj
                        col = h * 16 + i
                        mn = small.tile([128, 128], F16, name="mn")
                        nc.vector.tensor_scalar(
                            mn[:], dtiles[h][i][:], mus[:, col:col + 1],
                            rstd[:, col:col + 1], OP.subtract, OP.mult)
                        nc.tensor.transpose(
                            mt_ps[:, qj * 128:(qj + 1) * 128], mn[:],
                            ident_h[:])
                    nc.vector.tensor_copy(
                        MT[h][:, qb * 512:(qb + 1) * 512], mt_ps[:])

            # ---------------- phase C: out-proj ----------------
            with ExitStack() as cctx:
                ypool = cctx.enter_context(tc.tile_pool(name="ypool", bufs=2))
                mt_r = [MT[h][:].rearrange("p (m g) -> p g m", g=16)
                        for h in range(H_LOC)]
                for do in range(4):
                    if do not in wo_tiles:
                        wo_dma(do)()
                    wo_t = wo_tiles.pop(do)
                    if do + 2 < 4:
                        wo_dma(do + 2)()
                    ys = [ps.tile([128, 512], F32, name="ps")
                          for _ in range(H_LOC)]
                    for j in range(KD):
                        for h in range(H_LOC):
                            nc.tensor.matmul(
                                ys[h][:], mt_r[h][:, j, :], wo_t[:, j, :],
                                start=(j == 0), stop=(j == KD - 1))
                    for h in range(H_LOC):
                        y_sb = ypool.tile([128, 512], F32, name="ysb")
                        nc.scalar.copy(y_sb[:], ys[h][:])
                        nc.scalar.dma_start(
                            y_d[h * 128:(h + 1) * 128,
                                do * 512:(do + 1) * 512],
                            y_sb[:])

    nc.compile()
    return nc


_NC_CACHE = None


def make_in_maps(x, Wq, Wk, Wv, Wo, lambda_q1, lambda_k1, lambda_q2,
                 lambda_k2):
    x2 = np.ascontiguousarray(
        np.asarray(x, np.float32).reshape(T, D_EMB))
    Wq = np.asarray(Wq, np.float32)
    Wk = np.asarray(Wk, np.float32)
    Wv = np.asarray(Wv, np.float32)
    Wo16 = np.ascontiguousarray(np.asarray(Wo, np.float32).astype(np.float16))

    lam = (math.exp(float(np.dot(np.asarray(lambda_q1, np.float64),
                                 np.asarray(lambda_k1, np.float64))))
           - math.exp(float(np.dot(np.asarray(lambda_q2, np.float64),
                                   np.asarray(lambda_k2, np.float64))))
           + LAMBDA_INIT)
    neglam = np.full((128, 1), -lam, dtype=np.float32)

    in_maps = []
    for m in range(N_CORES):
        sl = slice(256 * m, 256 * (m + 1))
        in_maps.append({
            "x": x2,
            "wq": np.ascontiguousarray(Wq[:, sl]),
            "wk": np.ascontiguousarray(Wk[:, sl]),
            "wv": np.ascontiguousarray(Wv[:, sl]),
            "wo": Wo16,
            "neglam": neglam,
        })
    return in_maps


def kernel(x, Wq, Wk, Wv, Wo, lambda_q1, lambda_k1, lambda_q2, lambda_k2):
    global _NC_CACHE
    if _NC_CACHE is None:
        _NC_CACHE = _build_program()
    nc = _NC_CACHE

    B = np.asarray(x).shape[0]
    in_maps = make_in_maps(x, Wq, Wk, Wv, Wo, lambda_q1, lambda_k1,
                           lambda_q2, lambda_k2)
    res = run_bass_kernel_spmd(nc, in_maps, list(range(N_CORES)))
    y = np.concatenate([res.results[m]["y"] for m in range(N_CORES)], axis=0)
    return y.reshape(B, T, D_EMB)
